# revision 13
# baseline (speedup 1.0000x reference)
"""Trainium2 Bass kernel for nn_FreqCrossAttention.

Sharding: 8 cores = 4 batches x 2 head-groups (8 heads each).
Each core computes a partial output [2048, 1024] (its head-group's
contribution through W_o row-parallel); host sums the pair per batch.

DFT e-split: the two cores of a batch pair each compute the rfft of
only half the embedding columns (even core: e[0:512), odd core:
e[512:1024) — the odd core's q input is column-rotated so "local" is
always cols 0:512; LN stats are permutation-invariant).  The halves
are exchanged per frequency-chunk with a pair AllGather whose
rank-ordered output equals global e-order on both cores.

Pipeline per core (matmul operands bf16):
  kv path emitted first so the PE starts immediately; LN(q) runs
  concurrently on DVE/ACT.  DFT via matmul with cos/sin matrices
  (F padded to 1026), QKV projections in feature-major layout,
  Q/K/V assembled directly in SBUF (no DRAM bounce),
  scoresT = Kcat^T-blocks @ Qcat -> exp on ACT (scale=1/8, no max-sub)
  AV: expT blocks stationary, V||ones moving -> out + sumexp
  normalize, iDFT via matmul, W_o partial.
"""
import math
import numpy as np
import ml_dtypes

MM_BF16 = True
SPLIT_E = False       # pair-split DFT + AllGather: measured slower (pair
                      # AllGather ~40GB/s + latency > PE time saved)

B, L, E, H = 4, 2048, 1024, 16
D = E // H            # 64
Lf = L // 2 + 1       # 1025
FP = 1026             # padded frequency dim
NH = 8                # heads per core
P = 128
FCH = [(0, 384), (384, 384), (768, 258)]   # F chunks
LCH = [(0, 512), (512, 384), (896, 130)]   # l chunks for scores/exp
FTI = [(i * P, P) for i in range(8)] + [(1024, 2)]
MTI = [(i * P, P) for i in range(8)] + [(1024, 1)]
ET = 8                # e-chunks of E
ET_LOC = 4 if SPLIT_E else 8
E_LOC = ET_LOC * P
LT = 16               # L tiles
EPS = 1e-5
GROUPS = [[0, 1], [2, 3], [4, 5], [6, 7]]

_CACHE = {}


def _dft_consts():
    f = np.arange(FP)
    t = np.arange(L)
    ang = 2.0 * np.pi * np.outer(t, f) / L            # [L, FP]
    s = 1.0 / math.sqrt(L)
    FcT = (np.cos(ang) * s).astype(np.float32)        # rhs for rfft [L, FP]
    FsT = (-np.sin(ang) * s).astype(np.float32)
    FcT[:, Lf:] = 0.0
    FsT[:, Lf:] = 0.0
    cw = np.where((f == 0) | (f == L // 2), 1.0, 2.0)[:, None]
    GcT = (cw * np.cos(ang.T) * s).astype(np.float32)  # [FP, L]
    GsT = (-cw * np.sin(ang.T) * s).astype(np.float32)
    GcT[Lf:, :] = 0.0
    GsT[Lf:, :] = 0.0
    return FcT, FsT, GcT, GsT


def _build():
    import concourse.bass as bass
    import concourse.bacc as bacc
    import concourse.mybir as mybir
    import concourse.tile as tile

    R = mybir.dt.bfloat16 if MM_BF16 else mybir.dt.float32r
    F32 = mybir.dt.float32
    BF16 = mybir.dt.bfloat16
    AF = mybir.ActivationFunctionType

    nc = bacc.Bacc("TRN2", debug=False, num_devices=8)

    q_d = nc.dram_tensor("q", [L, E], F32, kind="ExternalInput")
    kv_d = nc.dram_tensor("kv", [L, E_LOC], R, kind="ExternalInput")
    gamma_d = nc.dram_tensor("gamma", [E_LOC, 1], F32, kind="ExternalInput")
    beta_d = nc.dram_tensor("beta", [E_LOC, 1], F32, kind="ExternalInput")
    FcT_d = nc.dram_tensor("FcT", [L, FP], R, kind="ExternalInput")
    FsT_d = nc.dram_tensor("FsT", [L, FP], R, kind="ExternalInput")
    GcT_d = nc.dram_tensor("GcT", [FP, L], R, kind="ExternalInput")
    GsT_d = nc.dram_tensor("GsT", [FP, L], R, kind="ExternalInput")
    W_d = {}
    for nm in ("qr", "qi", "kr", "ki", "vr", "vi"):
        W_d[nm] = nc.dram_tensor(f"W{nm}", [E, 512], R, kind="ExternalInput")
        W_d["b" + nm] = nc.dram_tensor(f"b{nm}", [512, 1], F32, kind="ExternalInput")
    WoT_d = nc.dram_tensor("WoT", [512, E], R, kind="ExternalInput")
    out_d = nc.dram_tensor("out", [L, E], F32, kind="ExternalOutput")

    with tile.TileContext(nc) as tc:
        with tc.tile_pool(name="dram", bufs=1, space="DRAM") as dram, \
             tc.tile_pool(name="dramg", bufs=2, space="DRAM") as dramg, \
             tc.tile_pool(name="persist", bufs=1) as persist, \
             tc.tile_pool(name="qkv", bufs=1) as qkv:
            qn_dram = dram.tile([L, E_LOC], R)

            # persistent SBUF destinations for attention operands
            Qc = [qkv.tile([P, FP], R, tag=f"Qc{h}", name=f"Qc{h}") for h in range(NH)]
            Kc = [qkv.tile([P, FP], R, tag=f"Kc{h}", name=f"Kc{h}") for h in range(NH)]
            Vc = [qkv.tile([P, NH * 129], BF16, tag=f"Vc{t}", name=f"Vc{t}")
                  for t in range(len(MTI))]

            # small persistent constants
            eps_t = persist.tile([P, 1], F32)
            nc.vector.memset(eps_t[:], EPS)
            gam = []
            bet = []
            for eb in range(ET_LOC):
                g = persist.tile([P, 1], F32, tag=f"gam{eb}", name=f"gam{eb}")
                nc.sync.dma_start(g[:], gamma_d.ap()[eb * P:(eb + 1) * P, :])
                gam.append(g)
                bt_ = persist.tile([P, 1], F32, tag=f"bet{eb}", name=f"bet{eb}")
                nc.sync.dma_start(bt_[:], beta_d.ap()[eb * P:(eb + 1) * P, :])
                bet.append(bt_)

            with tc.tile_pool(name="xep", bufs=1) as xep, \
                 tc.tile_pool(name="slab", bufs=2) as slab, \
                 tc.tile_pool(name="qf", bufs=1) as qfp, \
                 tc.tile_pool(name="lqf", bufs=1) as lqfp, \
                 tc.tile_pool(name="dps", bufs=2, space="PSUM") as dps, \
                 tc.tile_pool(name="stg", bufs=4) as stg:

                # ---- prefetch: kv x-data + chunk-0 DFT slab (gates first matmul)
                # interleaved per-lc so the eb0 accumulation group can start
                # streaming as soon as each lc block lands
                xe_kv = xep.tile([P, LT * E_LOC], R, tag="xe", name="xe")
                f0_0, fsz_0 = FCH[0]
                fct0 = slab.tile([P, LT * 384], R, tag="fct", name="fct")
                fst0 = slab.tile([P, LT * 384], R, tag="fst", name="fst")
                for lc in range(LT):
                    nc.sync.dma_start(xe_kv[:, lc * E_LOC:(lc + 1) * E_LOC],
                                      kv_d.ap()[lc * P:(lc + 1) * P, :])
                    nc.sync.dma_start(fct0[:, lc * fsz_0:(lc + 1) * fsz_0],
                                      FcT_d.ap()[lc * P:(lc + 1) * P, f0_0:f0_0 + fsz_0])
                    nc.sync.dma_start(fst0[:, lc * fsz_0:(lc + 1) * fsz_0],
                                      FsT_d.ap()[lc * P:(lc + 1) * P, f0_0:f0_0 + fsz_0])

                # kv-path weights on the ACT hwdge queue, ahead of the LN q
                # loads: needed by the chunk-0 projections at ~45us
                kv_wp = {}
                for nm in ("kr", "ki", "vr", "vi"):
                    kv_wp[nm] = [persist.tile([P, 512], R, tag=f"W{nm}{ec}", name=f"W{nm}{ec}") for ec in range(ET)]
                    for ec in range(ET):
                        nc.scalar.dma_start(kv_wp[nm][ec][:], W_d[nm].ap()[ec * P:(ec + 1) * P, :])

                # ---------------- Phase LN: qn = LN(q) -> qn_dram ----------------
                # Emitted after kv prefetch: runs on DVE/ACT while PE does kv DFT.
                with tc.tile_pool(name="ln", bufs=2) as ln, \
                     tc.tile_pool(name="lns", bufs=4) as lns:
                    for lc in range(LT):
                        qt = ln.tile([P, E], F32, tag="qt", name="qt")
                        nc.scalar.dma_start(qt[:], q_d.ap()[lc * P:(lc + 1) * P, :])
                        st = lns.tile([P, 12], F32, tag="st", name="st")
                        nc.vector.bn_stats(st[:, 0:6], qt[:, 0:512])
                        nc.vector.bn_stats(st[:, 6:12], qt[:, 512:1024])
                        mv = lns.tile([P, 2], F32, tag="mv", name="mv")
                        nc.vector.bn_aggr(mv[:], st[:])
                        sd = lns.tile([P, 1], F32, tag="sd", name="sd")
                        nc.scalar.activation(sd[:], mv[:, 1:2], AF.Sqrt, bias=eps_t[:])
                        istd = lns.tile([P, 1], F32, tag="istd", name="istd")
                        nc.vector.reciprocal(istd[:], sd[:])
                        nmu = lns.tile([P, 1], F32, tag="nmu", name="nmu")
                        nc.vector.tensor_scalar_mul(nmu[:], mv[:, 0:1], -1.0)
                        nc.vector.tensor_mul(nmu[:], nmu[:], istd[:])
                        qnt = ln.tile([P, E_LOC], R, tag="qnt", name="qnt")
                        nc.scalar.activation(qnt[:], qt[:, 0:E_LOC], AF.Identity,
                                             bias=nmu[:], scale=istd[:])
                        nc.scalar.dma_start(qn_dram[lc * P:(lc + 1) * P, :], qnt[:])

                # beta folded into DFT: qf_r[:, 0] += beta * sqrt(L)
                # gamma folded into DFT eviction as per-partition scale.

                def dft_proj(src_dram, is_q, xe=None, Wt_pre=None):
                    if xe is None:
                        xe = xep.tile([P, LT * E_LOC], R, tag="xe", name="xe")
                        for lc in range(LT):
                            nc.sync.dma_start(
                                xe[:, lc * E_LOC:(lc + 1) * E_LOC],
                                src_dram[lc * P:(lc + 1) * P, :] if is_q
                                else src_dram.ap()[lc * P:(lc + 1) * P, :])

                    with tc.tile_pool(name="wp", bufs=1) as wp:
                        names = ("qr", "qi") if is_q else ("kr", "ki", "vr", "vi")
                        Wt = {}
                        bias_t = {}
                        for nm in names:
                            if Wt_pre is not None:
                                Wt[nm] = Wt_pre[nm]
                            else:
                                Wt[nm] = [wp.tile([P, 512], R, tag=f"W{nm}{ec}", name=f"W{nm}{ec}") for ec in range(ET)]
                                for ec in range(ET):
                                    nc.scalar.dma_start(Wt[nm][ec][:], W_d[nm].ap()[ec * P:(ec + 1) * P, :])
                            if nm in ("qr", "qi", "kr", "ki"):
                                bias_t[nm] = [wp.tile([P, 1], F32, tag=f"b{nm}{mt}", name=f"b{nm}{mt}") for mt in range(4)]
                                for mt in range(4):
                                    nc.scalar.dma_start(bias_t[nm][mt][:],
                                                        W_d["b" + nm].ap()[mt * P:(mt + 1) * P, :])
                        vbias = None
                        if not is_q:
                            vb_row = wp.tile([1, 512], F32, tag="vbrow", name="vbrow")
                            vbias = {}
                            for nm in ("vr", "vi"):
                                nc.scalar.dma_start(vb_row[:], W_d["b" + nm].ap().rearrange("e one -> one e"))
                                vb = wp.tile([P, 512], F32, tag=f"vb{nm}", name=f"vb{nm}")
                                nc.gpsimd.partition_broadcast(vb[:], vb_row[:])
                                vbias[nm] = vb

                        cat = Qc if is_q else Kc

                        for fci, (f0, fsz) in enumerate(FCH):
                            if is_q or fci > 0:
                                fct = slab.tile([P, LT * 384], R, tag="fct", name="fct")
                                fst = slab.tile([P, LT * 384], R, tag="fst", name="fst")
                                for lc in range(LT):
                                    nc.sync.dma_start(fct[:, lc * fsz:(lc + 1) * fsz],
                                                      FcT_d.ap()[lc * P:(lc + 1) * P, f0:f0 + fsz])
                                    nc.sync.dma_start(fst[:, lc * fsz:(lc + 1) * fsz],
                                                      FsT_d.ap()[lc * P:(lc + 1) * P, f0:f0 + fsz])
                            else:
                                fct, fst = fct0, fst0

                            # local-half DFT
                            lfr = []
                            lfi = []
                            for eb in range(ET_LOC):
                                pr = dps.tile([P, 512], F32, tag="A", name="pA")
                                pi = dps.tile([P, 512], F32, tag="B", name="pB")
                                for lc in range(LT):
                                    xs = xe[:, lc * E_LOC + eb * P:lc * E_LOC + (eb + 1) * P]
                                    nc.tensor.matmul(pr[:, 0:fsz], xs,
                                                     fct[:, lc * fsz:(lc + 1) * fsz],
                                                     start=(lc == 0), stop=(lc == LT - 1))
                                    nc.tensor.matmul(pi[:, 0:fsz], xs,
                                                     fst[:, lc * fsz:(lc + 1) * fsz],
                                                     start=(lc == 0), stop=(lc == LT - 1))
                                fr = lqfp.tile([P, 384], R, tag=f"lfr{eb}", name=f"lfr{eb}")
                                fi = lqfp.tile([P, 384], R, tag=f"lfi{eb}", name=f"lfi{eb}")
                                if is_q:
                                    nc.scalar.activation(fr[:, 0:fsz], pr[:, 0:fsz], AF.Identity,
                                                         scale=gam[eb][:])
                                    nc.scalar.activation(fi[:, 0:fsz], pi[:, 0:fsz], AF.Identity,
                                                         scale=gam[eb][:])
                                    if fci == 0:
                                        # beta contributes only to DC (f=0)
                                        nc.vector.scalar_tensor_tensor(
                                            fr[:, 0:1], bet[eb][:], math.sqrt(L),
                                            fr[:, 0:1],
                                            op0=mybir.AluOpType.mult,
                                            op1=mybir.AluOpType.add)
                                else:
                                    nc.vector.tensor_copy(fr[:, 0:fsz], pr[:, 0:fsz])
                                    nc.vector.tensor_copy(fi[:, 0:fsz], pi[:, 0:fsz])
                                lfr.append(fr)
                                lfi.append(fi)

                            if SPLIT_E:
                                # exchange halves: pair AllGather, rank order == e order
                                gin = dramg.tile([2 * ET_LOC, P, 384], R, tag="gin", name="gin")
                                gout = dramg.tile([4 * ET_LOC, P, 384], R, tag="gout", name="gout")
                                for eb in range(ET_LOC):
                                    nc.sync.dma_start(gin[2 * eb, :, 0:fsz], lfr[eb][:, 0:fsz])
                                    nc.sync.dma_start(gin[2 * eb + 1, :, 0:fsz], lfi[eb][:, 0:fsz])
                                nc.gpsimd.collective_compute(
                                    "AllGather",
                                    mybir.AluOpType.bypass,
                                    replica_groups=GROUPS,
                                    ins=[gin[:].opt()],
                                    outs=[gout[:].opt()],
                                )
                                xfr = []
                                xfi = []
                                for g in range(ET):
                                    fr = qfp.tile([P, 384], R, tag=f"fr{g}", name=f"fr{g}")
                                    fi = qfp.tile([P, 384], R, tag=f"fi{g}", name=f"fi{g}")
                                    nc.sync.dma_start(fr[:, 0:fsz], gout[2 * g, :, 0:fsz])
                                    nc.sync.dma_start(fi[:, 0:fsz], gout[2 * g + 1, :, 0:fsz])
                                    xfr.append(fr)
                                    xfi.append(fi)
                            else:
                                xfr = lfr
                                xfi = lfi

                            # ---- Q/K projections for this fc block ----
                            pnames = ("qr", "qi") if is_q else ("kr", "ki")
                            for mt in range(4):
                                pps = {pnames[0]: dps.tile([P, 512], F32, tag="C", name="pC"),
                                       pnames[1]: dps.tile([P, 512], F32, tag="D", name="pD")}
                                for ec in range(ET):
                                    src = {pnames[0]: xfr[ec], pnames[1]: xfi[ec]}
                                    for nm in pnames:
                                        nc.tensor.matmul(pps[nm][:, 0:fsz],
                                                         Wt[nm][ec][:, mt * P:(mt + 1) * P],
                                                         src[nm][:, 0:fsz],
                                                         start=(ec == 0), stop=(ec == ET - 1))
                                sg = {}
                                for nm in pnames:
                                    s = stg.tile([P, 384], R, tag=f"sg{nm}", name=f"sg{nm}")
                                    nc.scalar.activation(s[:, 0:fsz], pps[nm][:, 0:fsz], AF.Identity,
                                                         bias=bias_t[nm][mt][:])
                                    sg[nm] = s
                                r0, i0 = pnames
                                nc.sync.dma_start(cat[2 * mt][0:64, f0:f0 + fsz], sg[r0][0:64, 0:fsz])
                                nc.sync.dma_start(cat[2 * mt + 1][0:64, f0:f0 + fsz], sg[r0][64:128, 0:fsz])
                                nc.sync.dma_start(cat[2 * mt][64:128, f0:f0 + fsz], sg[i0][0:64, 0:fsz])
                                nc.sync.dma_start(cat[2 * mt + 1][64:128, f0:f0 + fsz], sg[i0][64:128, 0:fsz])

                            # ---- V projection (kv path only): rows m in fc block ----
                            if not is_q:
                                for ti, (m0, msz) in enumerate(MTI):
                                    if not (m0 >= f0 and m0 + msz <= f0 + fsz):
                                        continue
                                    mr = m0 - f0
                                    pvr = dps.tile([P, 512], F32, tag="A", name="pA")
                                    pvi = dps.tile([P, 512], F32, tag="B", name="pB")
                                    for ec in range(ET):
                                        nc.tensor.matmul(pvr[0:msz, :], xfr[ec][:, mr:mr + msz],
                                                         Wt["vr"][ec][:],
                                                         start=(ec == 0), stop=(ec == ET - 1))
                                        nc.tensor.matmul(pvi[0:msz, :], xfi[ec][:, mr:mr + msz],
                                                         Wt["vi"][ec][:],
                                                         start=(ec == 0), stop=(ec == ET - 1))
                                    vco = Vc[ti][0:msz, :].rearrange("p (h c) -> p h c", h=NH)
                                    nc.vector.tensor_add(
                                        vco[:, :, 0:64],
                                        pvr[0:msz, :].rearrange("p (h c) -> p h c", h=NH),
                                        vbias["vr"][0:msz, :].rearrange("p (h c) -> p h c", h=NH))
                                    nc.vector.tensor_add(
                                        vco[:, :, 64:128],
                                        pvi[0:msz, :].rearrange("p (h c) -> p h c", h=NH),
                                        vbias["vi"][0:msz, :].rearrange("p (h c) -> p h c", h=NH))
                                    nc.vector.memset(vco[:, :, 128:129], 1.0)

                dft_proj(kv_d, False, xe=xe_kv, Wt_pre=kv_wp)
                dft_proj(qn_dram, True)

            # ---------------- Phase C: attention ----------------
            oacc_ctx = tc.tile_pool(name="oacc", bufs=1)
            oacc = oacc_ctx.__enter__()
            # prefetch Wo weights during attention (space freed by path pools)
            WoT_t = [oacc.tile([P, E], R, tag=f"wo{i}", name=f"wo{i}") for i in range(4)]
            for ec in range(4):
                nc.scalar.dma_start(WoT_t[ec][:], WoT_d.ap()[ec * P:(ec + 1) * P, :])
            attn_ctx = [tc.tile_pool(name="expp", bufs=2),
                        tc.tile_pool(name="sps", bufs=4, space="PSUM"),
                        tc.tile_pool(name="avps", bufs=3, space="PSUM"),
                        tc.tile_pool(name="nrm", bufs=4)]
            expp, sps, avps, nrm = [c.__enter__() for c in attn_ctx]
            if True:
                our = []
                oui = []
                for ti in range(len(FTI)):
                    our.append(oacc.tile([P, 512], R, tag=f"our{ti}", name=f"our{ti}"))
                    oui.append(oacc.tile([P, 512], R, tag=f"oui{ti}", name=f"oui{ti}"))

                for h in range(NH):
                    expts = []
                    for ti, (m0, msz) in enumerate(MTI):
                        et_ = expp.tile([P, FP], BF16, tag=f"exp{ti}", name=f"exp{ti}")
                        for (l0, lsz) in LCH:
                            ps = sps.tile([P, 512], F32, tag="sc", name="sc")
                            nc.tensor.matmul(ps[0:msz, 0:lsz], Kc[h][:, m0:m0 + msz],
                                             Qc[h][:, l0:l0 + lsz], start=True, stop=True)
                            nc.scalar.activation(et_[0:msz, l0:l0 + lsz], ps[0:msz, 0:lsz],
                                                 AF.Exp, scale=float(D ** -0.5))
                        expts.append(et_)
                    for ti, (l0, lsz) in enumerate(FTI):
                        ps = avps.tile([P, 129], F32, tag="av", name="av")
                        n = len(MTI)
                        for mi, (m0, msz) in enumerate(MTI):
                            nc.tensor.matmul(ps[0:lsz, :], expts[mi][0:msz, l0:l0 + lsz],
                                             Vc[mi][0:msz, h * 129:(h + 1) * 129],
                                             start=(mi == 0), stop=(mi == n - 1))
                        rcp = nrm.tile([P, 1], F32, tag="rcp", name="rcp")
                        nc.vector.reciprocal(rcp[0:lsz, :], ps[0:lsz, 128:129])
                        nc.vector.tensor_scalar_mul(our[ti][0:lsz, h * 64:(h + 1) * 64],
                                                    ps[0:lsz, 0:64], rcp[0:lsz, :])
                        nc.vector.tensor_scalar_mul(oui[ti][0:lsz, h * 64:(h + 1) * 64],
                                                    ps[0:lsz, 64:128], rcp[0:lsz, :])

                # ---------------- Phase D: iDFT + Wo ----------------
                for c in reversed(attn_ctx):
                    c.__exit__(None, None, None)
                with tc.tile_pool(name="gsl", bufs=2) as gsl, \
                     tc.tile_pool(name="ott", bufs=1) as ottp, \
                     tc.tile_pool(name="ost", bufs=3) as ost:
                    OTT = [ottp.tile([P, L], R, tag=f"OTT{i}", name=f"OTT{i}") for i in range(4)]
                    for half in range(2):
                        t0 = half * 1024
                        ops_ctx = tc.tile_pool(name=f"ops{half}", bufs=1, space="PSUM")
                        ops = ops_ctx.__enter__()
                        pst = [[ops.tile([P, 512], F32, tag=f"ph{e4}_{t2}", name=f"ph{e4}_{t2}")
                                for t2 in range(2)] for e4 in range(4)]
                        for mi, (m0, msz) in enumerate(FTI):
                            gc = gsl.tile([P, 1024], R, tag="gc", name="gc")
                            gs = gsl.tile([P, 1024], R, tag="gs", name="gs")
                            nc.sync.dma_start(gc[0:msz, :], GcT_d.ap()[m0:m0 + msz, t0:t0 + 1024])
                            nc.sync.dma_start(gs[0:msz, :], GsT_d.ap()[m0:m0 + msz, t0:t0 + 1024])
                            n = len(FTI)
                            for e4 in range(4):
                                for t2 in range(2):
                                    nc.tensor.matmul(pst[e4][t2][:],
                                                     our[mi][0:msz, e4 * P:(e4 + 1) * P],
                                                     gc[0:msz, t2 * 512:(t2 + 1) * 512],
                                                     start=(mi == 0), stop=False)
                                    nc.tensor.matmul(pst[e4][t2][:],
                                                     oui[mi][0:msz, e4 * P:(e4 + 1) * P],
                                                     gs[0:msz, t2 * 512:(t2 + 1) * 512],
                                                     start=False, stop=(mi == n - 1))
                        for e4 in range(4):
                            for t2 in range(2):
                                nc.vector.tensor_copy(
                                    OTT[e4][:, t0 + t2 * 512:t0 + (t2 + 1) * 512],
                                    pst[e4][t2][:])
                        ops_ctx.__exit__(None, None, None)
                    wops_ctx = tc.tile_pool(name="wops", bufs=2, space="PSUM")
                    wops = wops_ctx.__enter__()
                    for tb in range(LT):
                        pso = [wops.tile([P, 512], F32, tag=f"po{eo}", name=f"po{eo}") for eo in range(2)]
                        for eo in range(2):
                            for ec in range(4):
                                nc.tensor.matmul(pso[eo][:],
                                                 OTT[ec][:, tb * P:(tb + 1) * P],
                                                 WoT_t[ec][:, eo * 512:(eo + 1) * 512],
                                                 start=(ec == 0), stop=(ec == 3))
                        ot_ = ost.tile([P, E], F32, tag="ot", name="ot")
                        for eo in range(2):
                            nc.vector.tensor_copy(ot_[:, eo * 512:(eo + 1) * 512], pso[eo][:])
                        nc.sync.dma_start(out_d.ap()[tb * P:(tb + 1) * P, :], ot_[:])
                    wops_ctx.__exit__(None, None, None)
                oacc_ctx.__exit__(None, None, None)

    nc.finalize()
    return nc


def kernel(**inputs):
    from concourse.bass_utils import run_bass_kernel_spmd

    if "nc" not in _CACHE:
        _CACHE["nc"] = _build()
        _CACHE["consts"] = _dft_consts()
    nc = _CACHE["nc"]
    FcT, FsT, GcT, GsT = _CACHE["consts"]

    rdt = ml_dtypes.bfloat16 if MM_BF16 else np.float32
    q = np.ascontiguousarray(inputs["query"], dtype=np.float32)
    kv = np.ascontiguousarray(inputs["key_value"], dtype=rdt)
    gamma = np.ascontiguousarray(inputs["gamma"], np.float32)
    beta = np.ascontiguousarray(inputs["beta"], np.float32)
    in_maps = []
    for core in range(8):
        b = core // 2
        hg = core % 2
        cs = slice(hg * 512, (hg + 1) * 512)
        if SPLIT_E:
            es = slice(hg * 512, (hg + 1) * 512)   # rank parity = e-half owner
            if hg == 0:
                q_in = q[b]
            else:
                q_in = np.concatenate([q[b][:, 512:], q[b][:, :512]], axis=1)
            kv_in = np.ascontiguousarray(kv[b][:, es])
            gamma_in = np.ascontiguousarray(gamma[es]).reshape(E_LOC, 1)
            beta_in = np.ascontiguousarray(beta[es]).reshape(E_LOC, 1)
        else:
            q_in = q[b]
            kv_in = np.ascontiguousarray(kv[b])
            gamma_in = gamma.reshape(E, 1)
            beta_in = beta.reshape(E, 1)
        m = {
            "q": np.ascontiguousarray(q_in),
            "kv": kv_in,
            "gamma": gamma_in,
            "beta": beta_in,
            "FcT": FcT.astype(rdt), "FsT": FsT.astype(rdt),
            "GcT": GcT.astype(rdt), "GsT": GsT.astype(rdt),
            "WoT": np.ascontiguousarray(inputs["Wo"][:, cs].T.astype(rdt)),
        }
        for nm in ("qr", "qi", "kr", "ki", "vr", "vi"):
            m[f"W{nm}"] = np.ascontiguousarray(inputs["W" + nm][cs, :].T.astype(rdt))
            m[f"b{nm}"] = np.ascontiguousarray(inputs["b" + nm][cs], np.float32).reshape(512, 1)
        in_maps.append(m)

    res = run_bass_kernel_spmd(nc, in_maps, core_ids=list(range(8)))
    _CACHE["last"] = res
    out = np.empty((B, L, E), np.float32)
    for b in range(B):
        out[b] = res.results[2 * b]["out"] + res.results[2 * b + 1]["out"]
    return out


# revision 14
# speedup vs baseline: 1.0926x; 1.0926x over previous
"""Trainium2 Bass kernel for nn_FreqCrossAttention.

Sharding: 8 cores = 4 batches x 2 head-groups (8 heads each).
Each core computes a partial output [2048, 1024] (its head-group's
contribution through W_o row-parallel); host sums the pair per batch.

DFT e-split: the two cores of a batch pair each compute the rfft of
only half the embedding columns (even core: e[0:512), odd core:
e[512:1024) — the odd core's q input is column-rotated so "local" is
always cols 0:512; LN stats are permutation-invariant).  The halves
are exchanged per frequency-chunk with a pair AllGather whose
rank-ordered output equals global e-order on both cores.

Pipeline per core (matmul operands bf16):
  kv path emitted first so the PE starts immediately; LN(q) runs
  concurrently on DVE/ACT.  DFT via matmul with cos/sin matrices
  (F padded to 1026), QKV projections in feature-major layout,
  Q/K/V assembled directly in SBUF (no DRAM bounce),
  scoresT = Kcat^T-blocks @ Qcat -> exp on ACT (scale=1/8, no max-sub)
  AV: expT blocks stationary, V||ones moving -> out + sumexp
  normalize, iDFT via matmul, W_o partial.
"""
import math
import numpy as np
import ml_dtypes

MM_BF16 = True
SPLIT_E = False       # pair-split DFT + AllGather: measured slower (pair
                      # AllGather ~40GB/s + latency > PE time saved)

B, L, E, H = 4, 2048, 1024, 16
D = E // H            # 64
Lf = L // 2 + 1       # 1025
FP = 1026             # padded frequency dim
NH = 8                # heads per core
P = 128
FCH = [(0, 384), (384, 384), (768, 258)]   # F chunks
LCH = [(0, 512), (512, 384), (896, 130)]   # l chunks for scores/exp
FTI = [(i * P, P) for i in range(8)] + [(1024, 2)]
MTI = [(i * P, P) for i in range(8)] + [(1024, 1)]
ET = 8                # e-chunks of E
ET_LOC = 4 if SPLIT_E else 8
E_LOC = ET_LOC * P
LT = 16               # L tiles
EPS = 1e-5
GROUPS = [[0, 1], [2, 3], [4, 5], [6, 7]]

_CACHE = {}


def _dft_consts():
    f = np.arange(FP)
    t = np.arange(L)
    ang = 2.0 * np.pi * np.outer(t, f) / L            # [L, FP]
    s = 1.0 / math.sqrt(L)
    FcT = (np.cos(ang) * s).astype(np.float32)        # rhs for rfft [L, FP]
    FsT = (-np.sin(ang) * s).astype(np.float32)
    FcT[:, Lf:] = 0.0
    FsT[:, Lf:] = 0.0
    cw = np.where((f == 0) | (f == L // 2), 1.0, 2.0)[:, None]
    GcT = (cw * np.cos(ang.T) * s).astype(np.float32)  # [FP, L]
    GsT = (-cw * np.sin(ang.T) * s).astype(np.float32)
    GcT[Lf:, :] = 0.0
    GsT[Lf:, :] = 0.0
    return FcT, FsT, GcT, GsT


def _build():
    import concourse.bass as bass
    import concourse.bacc as bacc
    import concourse.mybir as mybir
    import concourse.tile as tile

    R = mybir.dt.bfloat16 if MM_BF16 else mybir.dt.float32r
    F32 = mybir.dt.float32
    BF16 = mybir.dt.bfloat16
    AF = mybir.ActivationFunctionType

    nc = bacc.Bacc("TRN2", debug=False, num_devices=8)

    q_d = nc.dram_tensor("q", [L, E], F32, kind="ExternalInput")
    kv_d = nc.dram_tensor("kv", [L, E_LOC], R, kind="ExternalInput")
    gamma_d = nc.dram_tensor("gamma", [E_LOC, 1], F32, kind="ExternalInput")
    beta_d = nc.dram_tensor("beta", [E_LOC, 1], F32, kind="ExternalInput")
    FcT_d = nc.dram_tensor("FcT", [L, FP], R, kind="ExternalInput")
    FsT_d = nc.dram_tensor("FsT", [L, FP], R, kind="ExternalInput")
    GcT_d = nc.dram_tensor("GcT", [FP, L], R, kind="ExternalInput")
    GsT_d = nc.dram_tensor("GsT", [FP, L], R, kind="ExternalInput")
    W_d = {}
    for nm in ("qr", "qi", "kr", "ki", "vr", "vi"):
        W_d[nm] = nc.dram_tensor(f"W{nm}", [E, 512], R, kind="ExternalInput")
        W_d["b" + nm] = nc.dram_tensor(f"b{nm}", [512, 1], F32, kind="ExternalInput")
    WoT_d = nc.dram_tensor("WoT", [512, E], R, kind="ExternalInput")
    out_d = nc.dram_tensor("out", [L, E], F32, kind="ExternalOutput")

    with tile.TileContext(nc) as tc:
        with tc.tile_pool(name="dram", bufs=1, space="DRAM") as dram, \
             tc.tile_pool(name="dramg", bufs=2, space="DRAM") as dramg, \
             tc.tile_pool(name="persist", bufs=1) as persist, \
             tc.tile_pool(name="qkv", bufs=1) as qkv:
            qn_dram = dram.tile([L, E_LOC], R)

            # persistent SBUF destinations for attention operands
            Qc = [qkv.tile([P, FP], R, tag=f"Qc{h}", name=f"Qc{h}") for h in range(NH)]
            Kc = [qkv.tile([P, FP], R, tag=f"Kc{h}", name=f"Kc{h}") for h in range(NH)]
            Vc = [qkv.tile([P, NH * 129], BF16, tag=f"Vc{t}", name=f"Vc{t}")
                  for t in range(len(MTI))]

            # small persistent constants
            eps_t = persist.tile([P, 1], F32)
            nc.vector.memset(eps_t[:], EPS)
            gam = []
            bet = []
            for eb in range(ET_LOC):
                g = persist.tile([P, 1], F32, tag=f"gam{eb}", name=f"gam{eb}")
                nc.sync.dma_start(g[:], gamma_d.ap()[eb * P:(eb + 1) * P, :])
                gam.append(g)
                bt_ = persist.tile([P, 1], F32, tag=f"bet{eb}", name=f"bet{eb}")
                nc.sync.dma_start(bt_[:], beta_d.ap()[eb * P:(eb + 1) * P, :])
                bet.append(bt_)

            with tc.tile_pool(name="xep", bufs=1) as xep, \
                 tc.tile_pool(name="slab", bufs=2) as slab, \
                 tc.tile_pool(name="qf", bufs=1) as qfp, \
                 tc.tile_pool(name="lqf", bufs=1) as lqfp, \
                 tc.tile_pool(name="dps", bufs=2, space="PSUM") as dps, \
                 tc.tile_pool(name="stg", bufs=4) as stg:

                # ---- prefetch: kv x-data + chunk-0 DFT slab (gates first matmul)
                # interleaved per-lc so the eb0 accumulation group can start
                # streaming as soon as each lc block lands
                xe_kv = xep.tile([P, LT * E_LOC], R, tag="xe", name="xe")
                f0_0, fsz_0 = FCH[0]
                fct0 = slab.tile([P, LT * 384], R, tag="fct", name="fct")
                fst0 = slab.tile([P, LT * 384], R, tag="fst", name="fst")
                for lc in range(LT):
                    nc.sync.dma_start(xe_kv[:, lc * E_LOC:(lc + 1) * E_LOC],
                                      kv_d.ap()[lc * P:(lc + 1) * P, :])
                    nc.sync.dma_start(fct0[:, lc * fsz_0:(lc + 1) * fsz_0],
                                      FcT_d.ap()[lc * P:(lc + 1) * P, f0_0:f0_0 + fsz_0])
                    nc.sync.dma_start(fst0[:, lc * fsz_0:(lc + 1) * fsz_0],
                                      FsT_d.ap()[lc * P:(lc + 1) * P, f0_0:f0_0 + fsz_0])

                # kv-path weights on the ACT hwdge queue, ahead of the LN q
                # loads: needed by the chunk-0 projections at ~45us
                kv_wp = {}
                for nm in ("kr", "ki", "vr", "vi"):
                    kv_wp[nm] = [persist.tile([P, 512], R, tag=f"W{nm}{ec}", name=f"W{nm}{ec}") for ec in range(ET)]
                    for ec in range(ET):
                        nc.sync.dma_start(kv_wp[nm][ec][:], W_d[nm].ap()[ec * P:(ec + 1) * P, :])

                # ---------------- Phase LN: qn = LN(q) -> qn_dram ----------------
                # Emitted after kv prefetch: runs on DVE/ACT while PE does kv DFT.
                with tc.tile_pool(name="ln", bufs=2) as ln, \
                     tc.tile_pool(name="lns", bufs=4) as lns:
                    for lc in range(LT):
                        qt = ln.tile([P, E], F32, tag="qt", name="qt")
                        nc.gpsimd.dma_start(qt[:], q_d.ap()[lc * P:(lc + 1) * P, :])
                        st = lns.tile([P, 12], F32, tag="st", name="st")
                        nc.vector.bn_stats(st[:, 0:6], qt[:, 0:512])
                        nc.vector.bn_stats(st[:, 6:12], qt[:, 512:1024])
                        mv = lns.tile([P, 2], F32, tag="mv", name="mv")
                        nc.vector.bn_aggr(mv[:], st[:])
                        sd = lns.tile([P, 1], F32, tag="sd", name="sd")
                        nc.scalar.activation(sd[:], mv[:, 1:2], AF.Sqrt, bias=eps_t[:])
                        istd = lns.tile([P, 1], F32, tag="istd", name="istd")
                        nc.vector.reciprocal(istd[:], sd[:])
                        nmu = lns.tile([P, 1], F32, tag="nmu", name="nmu")
                        nc.vector.tensor_scalar_mul(nmu[:], mv[:, 0:1], -1.0)
                        nc.vector.tensor_mul(nmu[:], nmu[:], istd[:])
                        qnt = ln.tile([P, E_LOC], R, tag="qnt", name="qnt")
                        nc.scalar.activation(qnt[:], qt[:, 0:E_LOC], AF.Identity,
                                             bias=nmu[:], scale=istd[:])
                        nc.gpsimd.dma_start(qn_dram[lc * P:(lc + 1) * P, :], qnt[:])

                # beta folded into DFT: qf_r[:, 0] += beta * sqrt(L)
                # gamma folded into DFT eviction as per-partition scale.

                def dft_proj(src_dram, is_q, xe=None, Wt_pre=None):
                    if xe is None:
                        xe = xep.tile([P, LT * E_LOC], R, tag="xe", name="xe")
                        for lc in range(LT):
                            nc.sync.dma_start(
                                xe[:, lc * E_LOC:(lc + 1) * E_LOC],
                                src_dram[lc * P:(lc + 1) * P, :] if is_q
                                else src_dram.ap()[lc * P:(lc + 1) * P, :])

                    with tc.tile_pool(name="wp", bufs=1) as wp:
                        names = ("qr", "qi") if is_q else ("kr", "ki", "vr", "vi")
                        Wt = {}
                        bias_t = {}
                        for nm in names:
                            if Wt_pre is not None:
                                Wt[nm] = Wt_pre[nm]
                            else:
                                Wt[nm] = [wp.tile([P, 512], R, tag=f"W{nm}{ec}", name=f"W{nm}{ec}") for ec in range(ET)]
                                for ec in range(ET):
                                    nc.sync.dma_start(Wt[nm][ec][:], W_d[nm].ap()[ec * P:(ec + 1) * P, :])
                            if nm in ("qr", "qi", "kr", "ki"):
                                bias_t[nm] = [wp.tile([P, 1], F32, tag=f"b{nm}{mt}", name=f"b{nm}{mt}") for mt in range(4)]
                                for mt in range(4):
                                    nc.sync.dma_start(bias_t[nm][mt][:],
                                                        W_d["b" + nm].ap()[mt * P:(mt + 1) * P, :])
                        vbias = None
                        if not is_q:
                            vb_row = wp.tile([1, 512], F32, tag="vbrow", name="vbrow")
                            vbias = {}
                            for nm in ("vr", "vi"):
                                nc.sync.dma_start(vb_row[:], W_d["b" + nm].ap().rearrange("e one -> one e"))
                                vb = wp.tile([P, 512], F32, tag=f"vb{nm}", name=f"vb{nm}")
                                nc.gpsimd.partition_broadcast(vb[:], vb_row[:])
                                vbias[nm] = vb

                        cat = Qc if is_q else Kc

                        for fci, (f0, fsz) in enumerate(FCH):
                            if is_q or fci > 0:
                                fct = slab.tile([P, LT * 384], R, tag="fct", name="fct")
                                fst = slab.tile([P, LT * 384], R, tag="fst", name="fst")
                                for lc in range(LT):
                                    nc.sync.dma_start(fct[:, lc * fsz:(lc + 1) * fsz],
                                                      FcT_d.ap()[lc * P:(lc + 1) * P, f0:f0 + fsz])
                                    nc.sync.dma_start(fst[:, lc * fsz:(lc + 1) * fsz],
                                                      FsT_d.ap()[lc * P:(lc + 1) * P, f0:f0 + fsz])
                            else:
                                fct, fst = fct0, fst0

                            # local-half DFT
                            lfr = []
                            lfi = []
                            for eb in range(ET_LOC):
                                pr = dps.tile([P, 512], F32, tag="A", name="pA")
                                pi = dps.tile([P, 512], F32, tag="B", name="pB")
                                for lc in range(LT):
                                    xs = xe[:, lc * E_LOC + eb * P:lc * E_LOC + (eb + 1) * P]
                                    nc.tensor.matmul(pr[:, 0:fsz], xs,
                                                     fct[:, lc * fsz:(lc + 1) * fsz],
                                                     start=(lc == 0), stop=(lc == LT - 1))
                                    nc.tensor.matmul(pi[:, 0:fsz], xs,
                                                     fst[:, lc * fsz:(lc + 1) * fsz],
                                                     start=(lc == 0), stop=(lc == LT - 1))
                                fr = lqfp.tile([P, 384], R, tag=f"lfr{eb}", name=f"lfr{eb}")
                                fi = lqfp.tile([P, 384], R, tag=f"lfi{eb}", name=f"lfi{eb}")
                                if is_q:
                                    nc.scalar.activation(fr[:, 0:fsz], pr[:, 0:fsz], AF.Identity,
                                                         scale=gam[eb][:])
                                    nc.scalar.activation(fi[:, 0:fsz], pi[:, 0:fsz], AF.Identity,
                                                         scale=gam[eb][:])
                                    if fci == 0:
                                        # beta contributes only to DC (f=0)
                                        nc.vector.scalar_tensor_tensor(
                                            fr[:, 0:1], bet[eb][:], math.sqrt(L),
                                            fr[:, 0:1],
                                            op0=mybir.AluOpType.mult,
                                            op1=mybir.AluOpType.add)
                                else:
                                    nc.vector.tensor_copy(fr[:, 0:fsz], pr[:, 0:fsz])
                                    nc.vector.tensor_copy(fi[:, 0:fsz], pi[:, 0:fsz])
                                lfr.append(fr)
                                lfi.append(fi)

                            if SPLIT_E:
                                # exchange halves: pair AllGather, rank order == e order
                                gin = dramg.tile([2 * ET_LOC, P, 384], R, tag="gin", name="gin")
                                gout = dramg.tile([4 * ET_LOC, P, 384], R, tag="gout", name="gout")
                                for eb in range(ET_LOC):
                                    nc.sync.dma_start(gin[2 * eb, :, 0:fsz], lfr[eb][:, 0:fsz])
                                    nc.sync.dma_start(gin[2 * eb + 1, :, 0:fsz], lfi[eb][:, 0:fsz])
                                nc.gpsimd.collective_compute(
                                    "AllGather",
                                    mybir.AluOpType.bypass,
                                    replica_groups=GROUPS,
                                    ins=[gin[:].opt()],
                                    outs=[gout[:].opt()],
                                )
                                xfr = []
                                xfi = []
                                for g in range(ET):
                                    fr = qfp.tile([P, 384], R, tag=f"fr{g}", name=f"fr{g}")
                                    fi = qfp.tile([P, 384], R, tag=f"fi{g}", name=f"fi{g}")
                                    nc.sync.dma_start(fr[:, 0:fsz], gout[2 * g, :, 0:fsz])
                                    nc.sync.dma_start(fi[:, 0:fsz], gout[2 * g + 1, :, 0:fsz])
                                    xfr.append(fr)
                                    xfi.append(fi)
                            else:
                                xfr = lfr
                                xfi = lfi

                            # ---- Q/K projections for this fc block ----
                            pnames = ("qr", "qi") if is_q else ("kr", "ki")
                            for mt in range(4):
                                pps = {pnames[0]: dps.tile([P, 512], F32, tag="C", name="pC"),
                                       pnames[1]: dps.tile([P, 512], F32, tag="D", name="pD")}
                                for ec in range(ET):
                                    src = {pnames[0]: xfr[ec], pnames[1]: xfi[ec]}
                                    for nm in pnames:
                                        nc.tensor.matmul(pps[nm][:, 0:fsz],
                                                         Wt[nm][ec][:, mt * P:(mt + 1) * P],
                                                         src[nm][:, 0:fsz],
                                                         start=(ec == 0), stop=(ec == ET - 1))
                                sg = {}
                                for nm in pnames:
                                    s = stg.tile([P, 384], R, tag=f"sg{nm}", name=f"sg{nm}")
                                    nc.scalar.activation(s[:, 0:fsz], pps[nm][:, 0:fsz], AF.Identity,
                                                         bias=bias_t[nm][mt][:])
                                    sg[nm] = s
                                r0, i0 = pnames
                                nc.sync.dma_start(cat[2 * mt][0:64, f0:f0 + fsz], sg[r0][0:64, 0:fsz])
                                nc.sync.dma_start(cat[2 * mt + 1][0:64, f0:f0 + fsz], sg[r0][64:128, 0:fsz])
                                nc.sync.dma_start(cat[2 * mt][64:128, f0:f0 + fsz], sg[i0][0:64, 0:fsz])
                                nc.sync.dma_start(cat[2 * mt + 1][64:128, f0:f0 + fsz], sg[i0][64:128, 0:fsz])

                            # ---- V projection (kv path only): rows m in fc block ----
                            if not is_q:
                                for ti, (m0, msz) in enumerate(MTI):
                                    if not (m0 >= f0 and m0 + msz <= f0 + fsz):
                                        continue
                                    mr = m0 - f0
                                    pvr = dps.tile([P, 512], F32, tag="A", name="pA")
                                    pvi = dps.tile([P, 512], F32, tag="B", name="pB")
                                    for ec in range(ET):
                                        nc.tensor.matmul(pvr[0:msz, :], xfr[ec][:, mr:mr + msz],
                                                         Wt["vr"][ec][:],
                                                         start=(ec == 0), stop=(ec == ET - 1))
                                        nc.tensor.matmul(pvi[0:msz, :], xfi[ec][:, mr:mr + msz],
                                                         Wt["vi"][ec][:],
                                                         start=(ec == 0), stop=(ec == ET - 1))
                                    vco = Vc[ti][0:msz, :].rearrange("p (h c) -> p h c", h=NH)
                                    nc.vector.tensor_add(
                                        vco[:, :, 0:64],
                                        pvr[0:msz, :].rearrange("p (h c) -> p h c", h=NH),
                                        vbias["vr"][0:msz, :].rearrange("p (h c) -> p h c", h=NH))
                                    nc.vector.tensor_add(
                                        vco[:, :, 64:128],
                                        pvi[0:msz, :].rearrange("p (h c) -> p h c", h=NH),
                                        vbias["vi"][0:msz, :].rearrange("p (h c) -> p h c", h=NH))
                                    nc.vector.memset(vco[:, :, 128:129], 1.0)

                dft_proj(kv_d, False, xe=xe_kv, Wt_pre=kv_wp)
                dft_proj(qn_dram, True)

            # ---------------- Phase C: attention ----------------
            oacc_ctx = tc.tile_pool(name="oacc", bufs=1)
            oacc = oacc_ctx.__enter__()
            # prefetch Wo weights during attention (space freed by path pools)
            WoT_t = [oacc.tile([P, E], R, tag=f"wo{i}", name=f"wo{i}") for i in range(4)]
            for ec in range(4):
                nc.sync.dma_start(WoT_t[ec][:], WoT_d.ap()[ec * P:(ec + 1) * P, :])
            attn_ctx = [tc.tile_pool(name="expp", bufs=2),
                        tc.tile_pool(name="sps", bufs=4, space="PSUM"),
                        tc.tile_pool(name="avps", bufs=3, space="PSUM"),
                        tc.tile_pool(name="nrm", bufs=4)]
            expp, sps, avps, nrm = [c.__enter__() for c in attn_ctx]
            if True:
                our = []
                oui = []
                for ti in range(len(FTI)):
                    our.append(oacc.tile([P, 512], R, tag=f"our{ti}", name=f"our{ti}"))
                    oui.append(oacc.tile([P, 512], R, tag=f"oui{ti}", name=f"oui{ti}"))

                for h in range(NH):
                    expts = []
                    for ti, (m0, msz) in enumerate(MTI):
                        et_ = expp.tile([P, FP], BF16, tag=f"exp{ti}", name=f"exp{ti}")
                        for (l0, lsz) in LCH:
                            ps = sps.tile([P, 512], F32, tag="sc", name="sc")
                            nc.tensor.matmul(ps[0:msz, 0:lsz], Kc[h][:, m0:m0 + msz],
                                             Qc[h][:, l0:l0 + lsz], start=True, stop=True)
                            nc.scalar.activation(et_[0:msz, l0:l0 + lsz], ps[0:msz, 0:lsz],
                                                 AF.Exp, scale=float(D ** -0.5))
                        expts.append(et_)
                    for ti, (l0, lsz) in enumerate(FTI):
                        ps = avps.tile([P, 129], F32, tag="av", name="av")
                        n = len(MTI)
                        for mi, (m0, msz) in enumerate(MTI):
                            nc.tensor.matmul(ps[0:lsz, :], expts[mi][0:msz, l0:l0 + lsz],
                                             Vc[mi][0:msz, h * 129:(h + 1) * 129],
                                             start=(mi == 0), stop=(mi == n - 1))
                        rcp = nrm.tile([P, 1], F32, tag="rcp", name="rcp")
                        nc.vector.reciprocal(rcp[0:lsz, :], ps[0:lsz, 128:129])
                        nc.vector.tensor_scalar_mul(our[ti][0:lsz, h * 64:(h + 1) * 64],
                                                    ps[0:lsz, 0:64], rcp[0:lsz, :])
                        nc.vector.tensor_scalar_mul(oui[ti][0:lsz, h * 64:(h + 1) * 64],
                                                    ps[0:lsz, 64:128], rcp[0:lsz, :])

                # ---------------- Phase D: iDFT + Wo ----------------
                for c in reversed(attn_ctx):
                    c.__exit__(None, None, None)
                with tc.tile_pool(name="gsl", bufs=2) as gsl, \
                     tc.tile_pool(name="ott", bufs=1) as ottp, \
                     tc.tile_pool(name="ost", bufs=3) as ost:
                    OTT = [ottp.tile([P, L], R, tag=f"OTT{i}", name=f"OTT{i}") for i in range(4)]
                    for half in range(2):
                        t0 = half * 1024
                        ops_ctx = tc.tile_pool(name=f"ops{half}", bufs=1, space="PSUM")
                        ops = ops_ctx.__enter__()
                        pst = [[ops.tile([P, 512], F32, tag=f"ph{e4}_{t2}", name=f"ph{e4}_{t2}")
                                for t2 in range(2)] for e4 in range(4)]
                        for mi, (m0, msz) in enumerate(FTI):
                            gc = gsl.tile([P, 1024], R, tag="gc", name="gc")
                            gs = gsl.tile([P, 1024], R, tag="gs", name="gs")
                            nc.sync.dma_start(gc[0:msz, :], GcT_d.ap()[m0:m0 + msz, t0:t0 + 1024])
                            nc.sync.dma_start(gs[0:msz, :], GsT_d.ap()[m0:m0 + msz, t0:t0 + 1024])
                            n = len(FTI)
                            for e4 in range(4):
                                for t2 in range(2):
                                    nc.tensor.matmul(pst[e4][t2][:],
                                                     our[mi][0:msz, e4 * P:(e4 + 1) * P],
                                                     gc[0:msz, t2 * 512:(t2 + 1) * 512],
                                                     start=(mi == 0), stop=False)
                                    nc.tensor.matmul(pst[e4][t2][:],
                                                     oui[mi][0:msz, e4 * P:(e4 + 1) * P],
                                                     gs[0:msz, t2 * 512:(t2 + 1) * 512],
                                                     start=False, stop=(mi == n - 1))
                        for e4 in range(4):
                            for t2 in range(2):
                                nc.vector.tensor_copy(
                                    OTT[e4][:, t0 + t2 * 512:t0 + (t2 + 1) * 512],
                                    pst[e4][t2][:])
                        ops_ctx.__exit__(None, None, None)
                    wops_ctx = tc.tile_pool(name="wops", bufs=2, space="PSUM")
                    wops = wops_ctx.__enter__()
                    for tb in range(LT):
                        pso = [wops.tile([P, 512], F32, tag=f"po{eo}", name=f"po{eo}") for eo in range(2)]
                        for eo in range(2):
                            for ec in range(4):
                                nc.tensor.matmul(pso[eo][:],
                                                 OTT[ec][:, tb * P:(tb + 1) * P],
                                                 WoT_t[ec][:, eo * 512:(eo + 1) * 512],
                                                 start=(ec == 0), stop=(ec == 3))
                        ot_ = ost.tile([P, E], F32, tag="ot", name="ot")
                        for eo in range(2):
                            nc.vector.tensor_copy(ot_[:, eo * 512:(eo + 1) * 512], pso[eo][:])
                        nc.sync.dma_start(out_d.ap()[tb * P:(tb + 1) * P, :], ot_[:])
                    wops_ctx.__exit__(None, None, None)
                oacc_ctx.__exit__(None, None, None)

    nc.finalize()
    return nc


def kernel(**inputs):
    from concourse.bass_utils import run_bass_kernel_spmd

    if "nc" not in _CACHE:
        _CACHE["nc"] = _build()
        _CACHE["consts"] = _dft_consts()
    nc = _CACHE["nc"]
    FcT, FsT, GcT, GsT = _CACHE["consts"]

    rdt = ml_dtypes.bfloat16 if MM_BF16 else np.float32
    q = np.ascontiguousarray(inputs["query"], dtype=np.float32)
    kv = np.ascontiguousarray(inputs["key_value"], dtype=rdt)
    gamma = np.ascontiguousarray(inputs["gamma"], np.float32)
    beta = np.ascontiguousarray(inputs["beta"], np.float32)
    in_maps = []
    for core in range(8):
        b = core // 2
        hg = core % 2
        cs = slice(hg * 512, (hg + 1) * 512)
        if SPLIT_E:
            es = slice(hg * 512, (hg + 1) * 512)   # rank parity = e-half owner
            if hg == 0:
                q_in = q[b]
            else:
                q_in = np.concatenate([q[b][:, 512:], q[b][:, :512]], axis=1)
            kv_in = np.ascontiguousarray(kv[b][:, es])
            gamma_in = np.ascontiguousarray(gamma[es]).reshape(E_LOC, 1)
            beta_in = np.ascontiguousarray(beta[es]).reshape(E_LOC, 1)
        else:
            q_in = q[b]
            kv_in = np.ascontiguousarray(kv[b])
            gamma_in = gamma.reshape(E, 1)
            beta_in = beta.reshape(E, 1)
        m = {
            "q": np.ascontiguousarray(q_in),
            "kv": kv_in,
            "gamma": gamma_in,
            "beta": beta_in,
            "FcT": FcT.astype(rdt), "FsT": FsT.astype(rdt),
            "GcT": GcT.astype(rdt), "GsT": GsT.astype(rdt),
            "WoT": np.ascontiguousarray(inputs["Wo"][:, cs].T.astype(rdt)),
        }
        for nm in ("qr", "qi", "kr", "ki", "vr", "vi"):
            m[f"W{nm}"] = np.ascontiguousarray(inputs["W" + nm][cs, :].T.astype(rdt))
            m[f"b{nm}"] = np.ascontiguousarray(inputs["b" + nm][cs], np.float32).reshape(512, 1)
        in_maps.append(m)

    res = run_bass_kernel_spmd(nc, in_maps, core_ids=list(range(8)))
    _CACHE["last"] = res
    out = np.empty((B, L, E), np.float32)
    for b in range(B):
        out[b] = res.results[2 * b]["out"] + res.results[2 * b + 1]["out"]
    return out


# revision 16
# speedup vs baseline: 1.1395x; 1.0429x over previous
"""Trainium2 Bass kernel for nn_FreqCrossAttention.

Sharding: 8 cores = 4 batches x 2 head-groups (8 heads each).
Each core computes a partial output [2048, 1024] (its head-group's
contribution through W_o row-parallel); host sums the pair per batch.

DFT e-split: the two cores of a batch pair each compute the rfft of
only half the embedding columns (even core: e[0:512), odd core:
e[512:1024) — the odd core's q input is column-rotated so "local" is
always cols 0:512; LN stats are permutation-invariant).  The halves
are exchanged per frequency-chunk with a pair AllGather whose
rank-ordered output equals global e-order on both cores.

Pipeline per core (matmul operands bf16):
  kv path emitted first so the PE starts immediately; LN(q) runs
  concurrently on DVE/ACT.  DFT via matmul with cos/sin matrices
  (F padded to 1026), QKV projections in feature-major layout,
  Q/K/V assembled directly in SBUF (no DRAM bounce),
  scoresT = Kcat^T-blocks @ Qcat -> exp on ACT (scale=1/8, no max-sub)
  AV: expT blocks stationary, V||ones moving -> out + sumexp
  normalize, iDFT via matmul, W_o partial.
"""
import math
import numpy as np
import ml_dtypes

MM_BF16 = True
SPLIT_E = False       # pair-split DFT + AllGather: measured slower (pair
                      # AllGather ~40GB/s + latency > PE time saved)

B, L, E, H = 4, 2048, 1024, 16
D = E // H            # 64
Lf = L // 2 + 1       # 1025
FP = 1026             # padded frequency dim
NH = 8                # heads per core
P = 128
FCH = [(0, 384), (384, 384), (768, 258)]   # F chunks
LCH = [(0, 512), (512, 384), (896, 130)]   # l chunks for scores/exp
FTI = [(i * P, P) for i in range(8)] + [(1024, 2)]
MTI = [(i * P, P) for i in range(8)] + [(1024, 1)]
ET = 8                # e-chunks of E
ET_LOC = 4 if SPLIT_E else 8
E_LOC = ET_LOC * P
LT = 16               # L tiles
EPS = 1e-5
GROUPS = [[0, 1], [2, 3], [4, 5], [6, 7]]

_CACHE = {}


def _dft_consts():
    f = np.arange(FP)
    t = np.arange(L)
    ang = 2.0 * np.pi * np.outer(t, f) / L            # [L, FP]
    s = 1.0 / math.sqrt(L)
    FcT = (np.cos(ang) * s).astype(np.float32)        # rhs for rfft [L, FP]
    FsT = (-np.sin(ang) * s).astype(np.float32)
    FcT[:, Lf:] = 0.0
    FsT[:, Lf:] = 0.0
    cw = np.where((f == 0) | (f == L // 2), 1.0, 2.0)[:, None]
    GcT = (cw * np.cos(ang.T) * s).astype(np.float32)  # [FP, L]
    GsT = (-cw * np.sin(ang.T) * s).astype(np.float32)
    GcT[Lf:, :] = 0.0
    GsT[Lf:, :] = 0.0
    return FcT, FsT, GcT, GsT


def _build():
    import concourse.bass as bass
    import concourse.bacc as bacc
    import concourse.mybir as mybir
    import concourse.tile as tile

    R = mybir.dt.bfloat16 if MM_BF16 else mybir.dt.float32r
    F32 = mybir.dt.float32
    BF16 = mybir.dt.bfloat16
    AF = mybir.ActivationFunctionType

    nc = bacc.Bacc("TRN2", debug=False, num_devices=8)

    q_d = nc.dram_tensor("q", [L, E], F32, kind="ExternalInput")
    kv_d = nc.dram_tensor("kv", [L, E_LOC], R, kind="ExternalInput")
    gamma_d = nc.dram_tensor("gamma", [E_LOC, 1], F32, kind="ExternalInput")
    beta_d = nc.dram_tensor("beta", [E_LOC, 1], F32, kind="ExternalInput")
    FcT_d = nc.dram_tensor("FcT", [L, FP], R, kind="ExternalInput")
    FsT_d = nc.dram_tensor("FsT", [L, FP], R, kind="ExternalInput")
    GcT_d = nc.dram_tensor("GcT", [FP, L], R, kind="ExternalInput")
    GsT_d = nc.dram_tensor("GsT", [FP, L], R, kind="ExternalInput")
    W_d = {}
    for nm in ("qr", "qi", "kr", "ki", "vr", "vi"):
        W_d[nm] = nc.dram_tensor(f"W{nm}", [E, 512], R, kind="ExternalInput")
        W_d["b" + nm] = nc.dram_tensor(f"b{nm}", [512, 1], F32, kind="ExternalInput")
    WoT_d = nc.dram_tensor("WoT", [512, E], R, kind="ExternalInput")
    out_d = nc.dram_tensor("out", [L, E], F32, kind="ExternalOutput")

    with tile.TileContext(nc) as tc:
        with tc.tile_pool(name="dram", bufs=1, space="DRAM") as dram, \
             tc.tile_pool(name="dramg", bufs=2, space="DRAM") as dramg, \
             tc.tile_pool(name="persist", bufs=1) as persist, \
             tc.tile_pool(name="qkv", bufs=1) as qkv:
            qn_dram = dram.tile([L, E_LOC], R)

            # persistent SBUF destinations for attention operands
            Qc = [qkv.tile([P, FP], R, tag=f"Qc{h}", name=f"Qc{h}") for h in range(NH)]
            Kc = [qkv.tile([P, FP], R, tag=f"Kc{h}", name=f"Kc{h}") for h in range(NH)]
            Vc = [qkv.tile([P, NH * 129], BF16, tag=f"Vc{t}", name=f"Vc{t}")
                  for t in range(len(MTI))]

            # small persistent constants
            eps_t = persist.tile([P, 1], F32)
            nc.vector.memset(eps_t[:], EPS)
            gam = []
            bet = []
            for eb in range(ET_LOC):
                g = persist.tile([P, 1], F32, tag=f"gam{eb}", name=f"gam{eb}")
                nc.sync.dma_start(g[:], gamma_d.ap()[eb * P:(eb + 1) * P, :])
                gam.append(g)
                bt_ = persist.tile([P, 1], F32, tag=f"bet{eb}", name=f"bet{eb}")
                nc.sync.dma_start(bt_[:], beta_d.ap()[eb * P:(eb + 1) * P, :])
                bet.append(bt_)

            with tc.tile_pool(name="xep", bufs=1) as xep, \
                 tc.tile_pool(name="slab", bufs=2) as slab, \
                 tc.tile_pool(name="qf", bufs=1) as qfp, \
                 tc.tile_pool(name="lqf", bufs=1) as lqfp, \
                 tc.tile_pool(name="dps", bufs=2, space="PSUM") as dps, \
                 tc.tile_pool(name="stg", bufs=4) as stg:

                # ---- prefetch: kv x-data + chunk-0 DFT slab (gates first matmul)
                # interleaved per-lc so the eb0 accumulation group can start
                # streaming as soon as each lc block lands
                xe_kv = xep.tile([P, LT * E_LOC], R, tag="xe", name="xe")
                f0_0, fsz_0 = FCH[0]
                fct0 = slab.tile([P, LT * 384], R, tag="fct", name="fct")
                fst0 = slab.tile([P, LT * 384], R, tag="fst", name="fst")
                for lc in range(LT):
                    nc.sync.dma_start(xe_kv[:, lc * E_LOC:(lc + 1) * E_LOC],
                                      kv_d.ap()[lc * P:(lc + 1) * P, :])
                    nc.sync.dma_start(fct0[:, lc * fsz_0:(lc + 1) * fsz_0],
                                      FcT_d.ap()[lc * P:(lc + 1) * P, f0_0:f0_0 + fsz_0])
                    nc.sync.dma_start(fst0[:, lc * fsz_0:(lc + 1) * fsz_0],
                                      FsT_d.ap()[lc * P:(lc + 1) * P, f0_0:f0_0 + fsz_0])

                # kv-path weights on the ACT hwdge queue, ahead of the LN q
                # loads: needed by the chunk-0 projections at ~45us
                kv_wp = {}
                for nm in ("kr", "ki", "vr", "vi"):
                    kv_wp[nm] = [persist.tile([P, 512], R, tag=f"W{nm}{ec}", name=f"W{nm}{ec}") for ec in range(ET)]
                    for ec in range(ET):
                        nc.sync.dma_start(kv_wp[nm][ec][:], W_d[nm].ap()[ec * P:(ec + 1) * P, :])

                # ---------------- Phase LN: qn = LN(q) -> qn_dram ----------------
                # Emitted after kv prefetch: runs on DVE/ACT while PE does kv DFT.
                with tc.tile_pool(name="ln", bufs=2) as ln, \
                     tc.tile_pool(name="lns", bufs=4) as lns:
                    for lc in range(LT):
                        qt = ln.tile([P, E], F32, tag="qt", name="qt")
                        nc.gpsimd.dma_start(qt[:], q_d.ap()[lc * P:(lc + 1) * P, :])
                        st = lns.tile([P, 12], F32, tag="st", name="st")
                        nc.vector.bn_stats(st[:, 0:6], qt[:, 0:512])
                        nc.vector.bn_stats(st[:, 6:12], qt[:, 512:1024])
                        mv = lns.tile([P, 2], F32, tag="mv", name="mv")
                        nc.vector.bn_aggr(mv[:], st[:])
                        sd = lns.tile([P, 1], F32, tag="sd", name="sd")
                        nc.scalar.activation(sd[:], mv[:, 1:2], AF.Sqrt, bias=eps_t[:])
                        istd = lns.tile([P, 1], F32, tag="istd", name="istd")
                        nc.vector.reciprocal(istd[:], sd[:])
                        nmu = lns.tile([P, 1], F32, tag="nmu", name="nmu")
                        nc.vector.tensor_scalar_mul(nmu[:], mv[:, 0:1], -1.0)
                        nc.vector.tensor_mul(nmu[:], nmu[:], istd[:])
                        qnt = ln.tile([P, E_LOC], R, tag="qnt", name="qnt")
                        nc.scalar.activation(qnt[:], qt[:, 0:E_LOC], AF.Identity,
                                             bias=nmu[:], scale=istd[:])
                        nc.gpsimd.dma_start(qn_dram[lc * P:(lc + 1) * P, :], qnt[:])

                # beta folded into DFT: qf_r[:, 0] += beta * sqrt(L)
                # gamma folded into DFT eviction as per-partition scale.

                def dft_proj(src_dram, is_q, xe=None, Wt_pre=None):
                    if xe is None:
                        xe = xep.tile([P, LT * E_LOC], R, tag="xe", name="xe")
                        for lc in range(LT):
                            nc.sync.dma_start(
                                xe[:, lc * E_LOC:(lc + 1) * E_LOC],
                                src_dram[lc * P:(lc + 1) * P, :] if is_q
                                else src_dram.ap()[lc * P:(lc + 1) * P, :])

                    with tc.tile_pool(name="wp", bufs=1) as wp:
                        names = ("qr", "qi") if is_q else ("kr", "ki", "vr", "vi")
                        Wt = {}
                        bias_t = {}
                        for nm in names:
                            if Wt_pre is not None:
                                Wt[nm] = Wt_pre[nm]
                            else:
                                Wt[nm] = [wp.tile([P, 512], R, tag=f"W{nm}{ec}", name=f"W{nm}{ec}") for ec in range(ET)]
                                for ec in range(ET):
                                    nc.sync.dma_start(Wt[nm][ec][:], W_d[nm].ap()[ec * P:(ec + 1) * P, :])
                            if nm in ("qr", "qi", "kr", "ki"):
                                bias_t[nm] = [wp.tile([P, 1], F32, tag=f"b{nm}{mt}", name=f"b{nm}{mt}") for mt in range(4)]
                                for mt in range(4):
                                    nc.sync.dma_start(bias_t[nm][mt][:],
                                                        W_d["b" + nm].ap()[mt * P:(mt + 1) * P, :])
                        vbias = None
                        if not is_q:
                            vb_row = wp.tile([1, 512], F32, tag="vbrow", name="vbrow")
                            vbias = {}
                            for nm in ("vr", "vi"):
                                nc.sync.dma_start(vb_row[:], W_d["b" + nm].ap().rearrange("e one -> one e"))
                                vb = wp.tile([P, 512], F32, tag=f"vb{nm}", name=f"vb{nm}")
                                nc.gpsimd.partition_broadcast(vb[:], vb_row[:])
                                vbias[nm] = vb

                        cat = Qc if is_q else Kc

                        for fci, (f0, fsz) in enumerate(FCH):
                            if is_q or fci > 0:
                                fct = slab.tile([P, LT * 384], R, tag="fct", name="fct")
                                fst = slab.tile([P, LT * 384], R, tag="fst", name="fst")
                                for lc in range(LT):
                                    nc.sync.dma_start(fct[:, lc * fsz:(lc + 1) * fsz],
                                                      FcT_d.ap()[lc * P:(lc + 1) * P, f0:f0 + fsz])
                                    nc.sync.dma_start(fst[:, lc * fsz:(lc + 1) * fsz],
                                                      FsT_d.ap()[lc * P:(lc + 1) * P, f0:f0 + fsz])
                            else:
                                fct, fst = fct0, fst0

                            # local-half DFT
                            lfr = []
                            lfi = []
                            for eb in range(ET_LOC):
                                pr = dps.tile([P, 512], F32, tag="A", name="pA")
                                pi = dps.tile([P, 512], F32, tag="B", name="pB")
                                for lc in range(LT):
                                    xs = xe[:, lc * E_LOC + eb * P:lc * E_LOC + (eb + 1) * P]
                                    nc.tensor.matmul(pr[:, 0:fsz], xs,
                                                     fct[:, lc * fsz:(lc + 1) * fsz],
                                                     start=(lc == 0), stop=(lc == LT - 1))
                                    nc.tensor.matmul(pi[:, 0:fsz], xs,
                                                     fst[:, lc * fsz:(lc + 1) * fsz],
                                                     start=(lc == 0), stop=(lc == LT - 1))
                                fr = lqfp.tile([P, 384], R, tag=f"lfr{eb}", name=f"lfr{eb}")
                                fi = lqfp.tile([P, 384], R, tag=f"lfi{eb}", name=f"lfi{eb}")
                                if is_q:
                                    nc.scalar.activation(fr[:, 0:fsz], pr[:, 0:fsz], AF.Identity,
                                                         scale=gam[eb][:])
                                    nc.scalar.activation(fi[:, 0:fsz], pi[:, 0:fsz], AF.Identity,
                                                         scale=gam[eb][:])
                                    if fci == 0:
                                        # beta contributes only to DC (f=0)
                                        nc.vector.scalar_tensor_tensor(
                                            fr[:, 0:1], bet[eb][:], math.sqrt(L),
                                            fr[:, 0:1],
                                            op0=mybir.AluOpType.mult,
                                            op1=mybir.AluOpType.add)
                                else:
                                    nc.vector.tensor_copy(fr[:, 0:fsz], pr[:, 0:fsz])
                                    nc.vector.tensor_copy(fi[:, 0:fsz], pi[:, 0:fsz])
                                lfr.append(fr)
                                lfi.append(fi)

                            if SPLIT_E:
                                # exchange halves: pair AllGather, rank order == e order
                                gin = dramg.tile([2 * ET_LOC, P, 384], R, tag="gin", name="gin")
                                gout = dramg.tile([4 * ET_LOC, P, 384], R, tag="gout", name="gout")
                                for eb in range(ET_LOC):
                                    nc.sync.dma_start(gin[2 * eb, :, 0:fsz], lfr[eb][:, 0:fsz])
                                    nc.sync.dma_start(gin[2 * eb + 1, :, 0:fsz], lfi[eb][:, 0:fsz])
                                nc.gpsimd.collective_compute(
                                    "AllGather",
                                    mybir.AluOpType.bypass,
                                    replica_groups=GROUPS,
                                    ins=[gin[:].opt()],
                                    outs=[gout[:].opt()],
                                )
                                xfr = []
                                xfi = []
                                for g in range(ET):
                                    fr = qfp.tile([P, 384], R, tag=f"fr{g}", name=f"fr{g}")
                                    fi = qfp.tile([P, 384], R, tag=f"fi{g}", name=f"fi{g}")
                                    nc.sync.dma_start(fr[:, 0:fsz], gout[2 * g, :, 0:fsz])
                                    nc.sync.dma_start(fi[:, 0:fsz], gout[2 * g + 1, :, 0:fsz])
                                    xfr.append(fr)
                                    xfi.append(fi)
                            else:
                                xfr = lfr
                                xfi = lfi

                            # ---- Q/K projections for this fc block ----
                            pnames = ("qr", "qi") if is_q else ("kr", "ki")
                            for mt in range(4):
                                pps = {pnames[0]: dps.tile([P, 512], F32, tag="C", name="pC"),
                                       pnames[1]: dps.tile([P, 512], F32, tag="D", name="pD")}
                                for ec in range(ET):
                                    src = {pnames[0]: xfr[ec], pnames[1]: xfi[ec]}
                                    for nm in pnames:
                                        nc.tensor.matmul(pps[nm][:, 0:fsz],
                                                         Wt[nm][ec][:, mt * P:(mt + 1) * P],
                                                         src[nm][:, 0:fsz],
                                                         start=(ec == 0), stop=(ec == ET - 1))
                                sg = {}
                                for nm in pnames:
                                    s = stg.tile([P, 384], R, tag=f"sg{nm}", name=f"sg{nm}")
                                    nc.scalar.activation(s[:, 0:fsz], pps[nm][:, 0:fsz], AF.Identity,
                                                         bias=bias_t[nm][mt][:])
                                    sg[nm] = s
                                r0, i0 = pnames
                                nc.sync.dma_start(cat[2 * mt][0:64, f0:f0 + fsz], sg[r0][0:64, 0:fsz])
                                nc.sync.dma_start(cat[2 * mt + 1][0:64, f0:f0 + fsz], sg[r0][64:128, 0:fsz])
                                nc.sync.dma_start(cat[2 * mt][64:128, f0:f0 + fsz], sg[i0][0:64, 0:fsz])
                                nc.sync.dma_start(cat[2 * mt + 1][64:128, f0:f0 + fsz], sg[i0][64:128, 0:fsz])

                            # ---- V projection (kv path only): rows m in fc block ----
                            if not is_q:
                                for ti, (m0, msz) in enumerate(MTI):
                                    if not (m0 >= f0 and m0 + msz <= f0 + fsz):
                                        continue
                                    mr = m0 - f0
                                    pvr = dps.tile([P, 512], F32, tag="A", name="pA")
                                    pvi = dps.tile([P, 512], F32, tag="B", name="pB")
                                    for ec in range(ET):
                                        nc.tensor.matmul(pvr[0:msz, :], xfr[ec][:, mr:mr + msz],
                                                         Wt["vr"][ec][:],
                                                         start=(ec == 0), stop=(ec == ET - 1))
                                        nc.tensor.matmul(pvi[0:msz, :], xfi[ec][:, mr:mr + msz],
                                                         Wt["vi"][ec][:],
                                                         start=(ec == 0), stop=(ec == ET - 1))
                                    vco = Vc[ti][0:msz, :].rearrange("p (h c) -> p h c", h=NH)
                                    nc.vector.tensor_add(
                                        vco[:, :, 0:64],
                                        pvr[0:msz, :].rearrange("p (h c) -> p h c", h=NH),
                                        vbias["vr"][0:msz, :].rearrange("p (h c) -> p h c", h=NH))
                                    nc.vector.tensor_add(
                                        vco[:, :, 64:128],
                                        pvi[0:msz, :].rearrange("p (h c) -> p h c", h=NH),
                                        vbias["vi"][0:msz, :].rearrange("p (h c) -> p h c", h=NH))
                                    nc.vector.memset(vco[:, :, 128:129], 1.0)

                dft_proj(kv_d, False, xe=xe_kv, Wt_pre=kv_wp)
                dft_proj(qn_dram, True)

            # ---------------- Phase C: attention ----------------
            oacc_ctx = tc.tile_pool(name="oacc", bufs=1)
            oacc = oacc_ctx.__enter__()
            # prefetch Wo weights during attention (space freed by path pools)
            WoT_t = [oacc.tile([P, E], R, tag=f"wo{i}", name=f"wo{i}") for i in range(4)]
            for ec in range(4):
                nc.sync.dma_start(WoT_t[ec][:], WoT_d.ap()[ec * P:(ec + 1) * P, :])
            attn_ctx = [tc.tile_pool(name="expp", bufs=3),
                        tc.tile_pool(name="sps", bufs=4, space="PSUM"),
                        tc.tile_pool(name="avps", bufs=4, space="PSUM"),
                        tc.tile_pool(name="nrm", bufs=4)]
            expp, sps, avps, nrm = [c.__enter__() for c in attn_ctx]
            if True:
                our = []
                oui = []
                for ti in range(len(FTI)):
                    our.append(oacc.tile([P, 512], R, tag=f"our{ti}", name=f"our{ti}"))
                    oui.append(oacc.tile([P, 512], R, tag=f"oui{ti}", name=f"oui{ti}"))

                for h in range(NH):
                    expts = []
                    for ti, (m0, msz) in enumerate(MTI):
                        et_ = expp.tile([P, FP], BF16, tag=f"exp{ti}", name=f"exp{ti}")
                        for (l0, lsz) in LCH:
                            ps = sps.tile([P, 512], F32, tag="sc", name="sc")
                            nc.tensor.matmul(ps[0:msz, 0:lsz], Kc[h][:, m0:m0 + msz],
                                             Qc[h][:, l0:l0 + lsz], start=True, stop=True)
                            nc.scalar.activation(et_[0:msz, l0:l0 + lsz], ps[0:msz, 0:lsz],
                                                 AF.Exp, scale=float(D ** -0.5))
                        expts.append(et_)
                    for ti, (l0, lsz) in enumerate(FTI):
                        ps = avps.tile([P, 129], F32, tag="av", name="av")
                        n = len(MTI)
                        for mi, (m0, msz) in enumerate(MTI):
                            nc.tensor.matmul(ps[0:lsz, :], expts[mi][0:msz, l0:l0 + lsz],
                                             Vc[mi][0:msz, h * 129:(h + 1) * 129],
                                             start=(mi == 0), stop=(mi == n - 1))
                        rcp = nrm.tile([P, 1], F32, tag="rcp", name="rcp")
                        nc.vector.reciprocal(rcp[0:lsz, :], ps[0:lsz, 128:129])
                        nc.vector.tensor_scalar_mul(our[ti][0:lsz, h * 64:(h + 1) * 64],
                                                    ps[0:lsz, 0:64], rcp[0:lsz, :])
                        nc.vector.tensor_scalar_mul(oui[ti][0:lsz, h * 64:(h + 1) * 64],
                                                    ps[0:lsz, 64:128], rcp[0:lsz, :])

                # ---------------- Phase D: iDFT + Wo ----------------
                for c in reversed(attn_ctx):
                    c.__exit__(None, None, None)
                with tc.tile_pool(name="gsl", bufs=2) as gsl, \
                     tc.tile_pool(name="ott", bufs=1) as ottp, \
                     tc.tile_pool(name="ost", bufs=3) as ost:
                    OTT = [ottp.tile([P, L], R, tag=f"OTT{i}", name=f"OTT{i}") for i in range(4)]
                    for half in range(2):
                        t0 = half * 1024
                        ops_ctx = tc.tile_pool(name=f"ops{half}", bufs=1, space="PSUM")
                        ops = ops_ctx.__enter__()
                        pst = [[ops.tile([P, 512], F32, tag=f"ph{e4}_{t2}", name=f"ph{e4}_{t2}")
                                for t2 in range(2)] for e4 in range(4)]
                        for mi, (m0, msz) in enumerate(FTI):
                            if m0 == 1024:
                                msz = 1   # row 1025 is zero in GcT/GsT
                            gc = gsl.tile([P, 1024], R, tag="gc", name="gc")
                            gs = gsl.tile([P, 1024], R, tag="gs", name="gs")
                            nc.sync.dma_start(gc[0:msz, :], GcT_d.ap()[m0:m0 + msz, t0:t0 + 1024])
                            nc.sync.dma_start(gs[0:msz, :], GsT_d.ap()[m0:m0 + msz, t0:t0 + 1024])
                            n = len(FTI)
                            for e4 in range(4):
                                for t2 in range(2):
                                    nc.tensor.matmul(pst[e4][t2][:],
                                                     our[mi][0:msz, e4 * P:(e4 + 1) * P],
                                                     gc[0:msz, t2 * 512:(t2 + 1) * 512],
                                                     start=(mi == 0), stop=False)
                                    nc.tensor.matmul(pst[e4][t2][:],
                                                     oui[mi][0:msz, e4 * P:(e4 + 1) * P],
                                                     gs[0:msz, t2 * 512:(t2 + 1) * 512],
                                                     start=False, stop=(mi == n - 1))
                        for e4 in range(4):
                            for t2 in range(2):
                                nc.vector.tensor_copy(
                                    OTT[e4][:, t0 + t2 * 512:t0 + (t2 + 1) * 512],
                                    pst[e4][t2][:])
                        ops_ctx.__exit__(None, None, None)
                    wops_ctx = tc.tile_pool(name="wops", bufs=2, space="PSUM")
                    wops = wops_ctx.__enter__()
                    for tb in range(LT):
                        pso = [wops.tile([P, 512], F32, tag=f"po{eo}", name=f"po{eo}") for eo in range(2)]
                        for eo in range(2):
                            for ec in range(4):
                                nc.tensor.matmul(pso[eo][:],
                                                 OTT[ec][:, tb * P:(tb + 1) * P],
                                                 WoT_t[ec][:, eo * 512:(eo + 1) * 512],
                                                 start=(ec == 0), stop=(ec == 3))
                        ot_ = ost.tile([P, E], F32, tag="ot", name="ot")
                        for eo in range(2):
                            nc.vector.tensor_copy(ot_[:, eo * 512:(eo + 1) * 512], pso[eo][:])
                        nc.sync.dma_start(out_d.ap()[tb * P:(tb + 1) * P, :], ot_[:])
                    wops_ctx.__exit__(None, None, None)
                oacc_ctx.__exit__(None, None, None)

    nc.finalize()
    return nc


def kernel(**inputs):
    from concourse.bass_utils import run_bass_kernel_spmd

    if "nc" not in _CACHE:
        _CACHE["nc"] = _build()
        _CACHE["consts"] = _dft_consts()
    nc = _CACHE["nc"]
    FcT, FsT, GcT, GsT = _CACHE["consts"]

    rdt = ml_dtypes.bfloat16 if MM_BF16 else np.float32
    q = np.ascontiguousarray(inputs["query"], dtype=np.float32)
    kv = np.ascontiguousarray(inputs["key_value"], dtype=rdt)
    gamma = np.ascontiguousarray(inputs["gamma"], np.float32)
    beta = np.ascontiguousarray(inputs["beta"], np.float32)
    in_maps = []
    for core in range(8):
        b = core // 2
        hg = core % 2
        cs = slice(hg * 512, (hg + 1) * 512)
        if SPLIT_E:
            es = slice(hg * 512, (hg + 1) * 512)   # rank parity = e-half owner
            if hg == 0:
                q_in = q[b]
            else:
                q_in = np.concatenate([q[b][:, 512:], q[b][:, :512]], axis=1)
            kv_in = np.ascontiguousarray(kv[b][:, es])
            gamma_in = np.ascontiguousarray(gamma[es]).reshape(E_LOC, 1)
            beta_in = np.ascontiguousarray(beta[es]).reshape(E_LOC, 1)
        else:
            q_in = q[b]
            kv_in = np.ascontiguousarray(kv[b])
            gamma_in = gamma.reshape(E, 1)
            beta_in = beta.reshape(E, 1)
        m = {
            "q": np.ascontiguousarray(q_in),
            "kv": kv_in,
            "gamma": gamma_in,
            "beta": beta_in,
            "FcT": FcT.astype(rdt), "FsT": FsT.astype(rdt),
            "GcT": GcT.astype(rdt), "GsT": GsT.astype(rdt),
            "WoT": np.ascontiguousarray(inputs["Wo"][:, cs].T.astype(rdt)),
        }
        for nm in ("qr", "qi", "kr", "ki", "vr", "vi"):
            m[f"W{nm}"] = np.ascontiguousarray(inputs["W" + nm][cs, :].T.astype(rdt))
            m[f"b{nm}"] = np.ascontiguousarray(inputs["b" + nm][cs], np.float32).reshape(512, 1)
        in_maps.append(m)

    res = run_bass_kernel_spmd(nc, in_maps, core_ids=list(range(8)))
    _CACHE["last"] = res
    out = np.empty((B, L, E), np.float32)
    for b in range(B):
        out[b] = res.results[2 * b]["out"] + res.results[2 * b + 1]["out"]
    return out
